# revision 35
# baseline (speedup 1.0000x reference)
"""Trainium2 Bass kernel for nn_GATsimple (4-layer GAT + graph readout).

Self-contained: takes the FULL inputs from setup_inputs(), shards across 8
NeuronCores internally (data-parallel over dst-node ranges after a host-side
degree-balanced node->core remap), runs a Bass/Tile kernel via
run_bass_kernel_spmd, and returns the FULL [128, 1] output.

v2 design (vs v1):
  - bf16 h table: row = [h (fo) | s_hi (4) | s_lo (4) | pad] (256B-multiple
    rows); scores moved exactly via hi/lo bf16 pairs.
  - one-hot St (d-expand) and S (segment-sum) matrices are static per edge
    instance -> precomputed on host, loaded to SBUF once, reused all layers.
  - self-loops removed from the edge list and folded into the node phase
    (local, exact f32).
  - host remap balances in-degree across (core, window) bins -> uniform
    T tiles/window everywhere.
  - all matmuls bf16 (x, waug, msg, one-hots); PSUM f32.
  - gathers via dma_gather prepare_only + trigger_dma, prepped ahead.
  - node phase software-pipelined one window behind the edge phase.

Per layer:
  A. node matmul h_aug[v] = x[v] @ [W | WA_hi | WA_lo]  (A = blockdiag of
     a_src/a_dst); build bf16 table rows + dwin (d hi/lo) + sd32 (f32 s,d).
  B. AllGather table -> h_glob; write dummy row (s = -100).
  C. per 128-dst-node window: trigger prepped gather of h[src] rows;
     d-expand via St matmul; e = exp(lrelu(s+d)) (f32, out bf16);
     msg = ee*h (bf16); segment-sum via S matmul accumulated in PSUM.
  D. node phase (one window delayed): add self-loop term, xp = msg/denom
     + bias, ELU, PE-transpose into xT (bf16) for the next layer; last
     layer: p = x4 . fcw_node, graph accumulation via one-hot G matmul.
Readout: per-core partial per-graph sums [128,1] -> AllGather -> reduce ->
per-core one-hot column select -> y slice [16,1]; driver adds fcb.
"""

import os
import sys

import ml_dtypes
import numpy as np

for _p in ("/opt/trn_rl_repo", "/root/.axon_site/_ro/trn_rl_repo"):
    if os.path.isdir(_p) and _p not in sys.path:
        sys.path.append(_p)

import concourse.bass as bass
import concourse.bacc as bacc
import concourse.mybir as mybir
import concourse.tile as tile
from concourse.bass_utils import run_bass_kernel_spmd

F32 = mybir.dt.float32
BF16 = mybir.dt.bfloat16
I16 = mybir.dt.int16

N_CORES = 8
HEADS = 4
NWA = 9  # windows per core in AllGather chunk A (rest go to chunk B)
DUMMY_S = -100.0  # dummy-source s_hi; exp(lrelu(-100+d)) ~ 2e-9


class Cfg:
    def __init__(self, n_nodes, npg, in_feat, layer_out, n_cores=N_CORES):
        assert n_nodes % n_cores == 0
        self.n_nodes = n_nodes
        self.npg = npg
        self.n_graphs = n_nodes // npg
        self.n_cores = n_cores
        self.npc = n_nodes // n_cores
        self.nblk = (self.npc + 127) // 128
        self.npc_pad = self.nblk * 128
        self.dummy = n_cores * self.npc_pad
        self.nrows = self.dummy + 1
        self.in_feat = in_feat
        self.layer_out = layer_out
        self.f_out = [HEADS * c for c in layer_out]
        self.f_in = [in_feat] + self.f_out[:-1]
        self.n_layers = len(layer_out)
        self.gpc = self.n_graphs // n_cores
        # bf16 table rows: fo + 8 useful, padded to a multiple of 128 elems
        self.rowp = [max(128, ((fo + 8 + 127) // 128) * 128) for fo in self.f_out]


def default_cfg():
    return Cfg(n_nodes=17024, npg=133, in_feat=64, layer_out=[128, 64, 32, 16])


# ------------------------------------------------------------ host preprocess


def balance_nodes(cfg, dst):
    """Assign nodes to (core, window) bins, balancing in-degree.

    Returns (assign[node] -> bin, T tiles/window). Bin b = core*nblk + win.
    Window nblk-1 of each core holds npc - (nblk-1)*128 nodes; others 128.
    """
    import heapq

    n = cfg.n_nodes
    deg = np.bincount(dst, minlength=n).astype(np.int64)
    nbins = cfg.n_cores * cfg.nblk
    cap = np.full(nbins, 128, dtype=np.int64)
    cap[cfg.nblk - 1 :: cfg.nblk] = cfg.npc - (cfg.nblk - 1) * 128
    assert cap.sum() == n
    order = np.argsort(-deg, kind="stable")
    load = np.zeros(nbins, dtype=np.int64)
    cnt = np.zeros(nbins, dtype=np.int64)
    assign = np.empty(n, dtype=np.int64)
    heap = [(0, b) for b in range(nbins)]
    heapq.heapify(heap)
    for v in order:
        while True:
            l, b = heapq.heappop(heap)
            if cnt[b] < cap[b]:
                break
            # bin full; drop it
        assign[v] = b
        cnt[b] += 1
        load[b] += deg[v]
        if cnt[b] < cap[b]:
            heapq.heappush(heap, (load[b], b))
    T = max(1, int(-(-load.max() // 128)))
    return assign, T


def preprocess(cfg, edge_index):
    """Host preprocessing: node remap + per-core edge buckets + one-hots.

    Returns (T, perm_pad[node]->padded_id, per_core list of dicts).
    """
    src = np.asarray(edge_index[0], dtype=np.int64)
    dst = np.asarray(edge_index[1], dtype=np.int64)
    assign, T = balance_nodes(cfg, dst)

    # slot assignment within each bin, in node order
    nbins = cfg.n_cores * cfg.nblk
    slot = np.empty(cfg.n_nodes, dtype=np.int64)
    fill = np.zeros(nbins, dtype=np.int64)
    for v in range(cfg.n_nodes):
        b = assign[v]
        slot[v] = fill[b]
        fill[b] += 1
    core = assign // cfg.nblk
    win = assign % cfg.nblk
    perm_pad = core * cfg.npc_pad + win * 128 + slot  # node -> padded id

    # table row id in the A/B-half AllGather layout: windows 0..NWA-1 of
    # each core land in half A (core-major blocks of NWA*128 rows), the rest
    # in half B (core-major blocks at offset n_cores*NWA*128)
    nwa = NWA * 128
    nwb = cfg.npc_pad - nwa
    local = win * 128 + slot
    tab_row = np.where(
        local < nwa,
        core * nwa + local,
        cfg.n_cores * nwa + core * nwb + (local - nwa),
    )
    src_tab = tab_row[src]
    dst_core = core[dst]
    dst_win = win[dst]
    dst_slot = slot[dst]

    per_core = []
    for c in range(cfg.n_cores):
        m = dst_core == c
        e_src = src_tab[m]
        e_win = dst_win[m]
        e_slot = dst_slot[m]
        order = np.argsort(e_win, kind="stable")
        e_src, e_win, e_slot = e_src[order], e_win[order], e_slot[order]
        counts = np.bincount(e_win, minlength=cfg.nblk)
        assert counts.max() <= T * 128, (counts.max(), T * 128)
        starts = np.concatenate([[0], np.cumsum(counts)])
        tot = T * 128
        gidx_cols, st_cols, s_cols = [], [], []
        for g in range(cfg.nblk):
            s0, s1 = starts[g], starts[g + 1]
            cnt = s1 - s0
            sp = np.full(tot, cfg.dummy, dtype=np.int64)
            dl = np.zeros(tot, dtype=np.int64)
            sp[:cnt] = e_src[s0:s1]
            dl[:cnt] = e_slot[s0:s1]
            wrap = sp.astype(np.int16).reshape(-1, 16).T  # [16, T*8]
            gidx_cols.append(np.tile(wrap, (8, 1)))  # [128, T*8]
            # one-hots per tile: St[p, t*128+e] = (dl[t*128+e] == p)
            dlt = dl.reshape(T, 128)
            oh = (dlt[:, None, :] == np.arange(128)[None, :, None])  # [T,128p,128e]
            st_cols.append(
                np.concatenate(list(oh), axis=1).astype(ml_dtypes.bfloat16)
            )  # [128, T*128]
            s_cols.append(
                np.concatenate(
                    [oh[t].T for t in range(T)], axis=1
                ).astype(ml_dtypes.bfloat16)
            )  # [128e-part, T*128n]
        per_core.append(
            dict(
                gidx=np.ascontiguousarray(np.concatenate(gidx_cols, axis=1)),
                st=np.ascontiguousarray(np.concatenate(st_cols, axis=1)),
                s=np.ascontiguousarray(np.concatenate(s_cols, axis=1)),
            )
        )
    return T, perm_pad, per_core


def make_waug(W, a_s, a_d):
    """[W | (W@A)_hi | (W@A)_lo] as bf16; A = blockdiag score vectors."""
    fin, fout = W.shape
    H, C = a_s.shape
    assert H * C == fout
    A = np.zeros((fout, 2 * H), dtype=np.float64)
    for h in range(H):
        A[h * C : (h + 1) * C, h] = a_s[h]
        A[h * C : (h + 1) * C, H + h] = a_d[h]
    wa = W.astype(np.float64) @ A  # [fin, 8]
    wa_hi = wa.astype(ml_dtypes.bfloat16)
    wa_lo = (wa - wa_hi.astype(np.float64)).astype(ml_dtypes.bfloat16)
    return np.ascontiguousarray(
        np.concatenate(
            [W.astype(ml_dtypes.bfloat16), wa_hi, wa_lo], axis=1
        )
    )  # [fin, fout+16] bf16


# ---------------------------------------------------------------- bass kernel


def build_kernel(cfg, T, dbg=False):
    nblk = cfg.nblk
    ttot = T * nblk
    nwa = NWA * 128                 # rows per core in chunk A
    nwb = cfg.npc_pad - nwa         # rows per core in chunk B
    nc = bacc.Bacc(
        "TRN2",
        target_bir_lowering=False,
        debug=False,
        num_devices=cfg.n_cores,
    )

    # ---- I/O declarations (all per-core)
    xT0_d = nc.dram_tensor("xT0", [cfg.in_feat, cfg.npc_pad], BF16, kind="ExternalInput")
    waug_d, bias_d = [], []
    for l in range(cfg.n_layers):
        waug_d.append(
            nc.dram_tensor(
                f"waug{l}", [cfg.f_in[l], cfg.f_out[l] + 16], BF16, kind="ExternalInput"
            )
        )
        bias_d.append(
            nc.dram_tensor(f"bias{l}", [128, cfg.f_out[l]], F32, kind="ExternalInput")
        )
    gidx_d = nc.dram_tensor("gidx", [128, 8 * ttot], I16, kind="ExternalInput")
    st_d = nc.dram_tensor("st", [128, 128 * ttot], BF16, kind="ExternalInput")
    ss_d = nc.dram_tensor("ss", [128, 128 * ttot], BF16, kind="ExternalInput")
    g_d = nc.dram_tensor("gmat", [128, 128 * nblk], F32, kind="ExternalInput")
    ysel_d = nc.dram_tensor("ysel", [128, cfg.gpc], F32, kind="ExternalInput")
    fcwn_d = nc.dram_tensor("fcwn", [cfg.npc_pad, 64], F32, kind="ExternalInput")
    y_d = nc.dram_tensor("y", [cfg.gpc, 1], F32, kind="ExternalOutput")

    # internal DRAM; table layout: [A: core-major nwa blocks | B: core-major
    # nwb blocks | dummy row]
    h_inA, h_inB, h_glob = [], [], []
    for l in range(cfg.n_layers):
        rp = cfg.rowp[l]
        h_inA.append(nc.dram_tensor(f"h_inA{l}", [nwa, rp], BF16))
        h_inB.append(nc.dram_tensor(f"h_inB{l}", [nwb, rp], BF16))
        h_glob.append(
            nc.dram_tensor(f"h_glob{l}", [cfg.nrows, rp], BF16, addr_space="Shared")
        )
    ypart_d = nc.dram_tensor("ypart", [128, 1], F32)
    yglob_d = nc.dram_tensor("yglob", [cfg.n_cores * 128, 1], F32, addr_space="Shared")
    use_dbg = os.environ.get("GAT_DBG", "0") == "1"
    if use_dbg:
        dbg_p = nc.dram_tensor("dbg_p", [128, cfg.nblk], F32, kind="ExternalOutput")
        dbg_ya = nc.dram_tensor("dbg_ya", [128, cfg.n_cores], F32, kind="ExternalOutput")
        dbg_hs = nc.dram_tensor("dbg_hs", [128, cfg.rowp[0]], F32, kind="ExternalOutput")
        dbg_x1 = nc.dram_tensor("dbg_x1", [128, cfg.f_out[0]], F32, kind="ExternalOutput")

    ident_c = nc.inline_tensor(np.eye(128, dtype=np.float32), name="ident_c")

    rg = [list(range(cfg.n_cores))]
    AG = "AllGather"
    use_prep = os.environ.get("GAT_PREP", "0") == "1"
    AHEAD = int(os.environ.get("GAT_AHEAD", "2"))

    with tile.TileContext(nc) as tc:
        with (
            tc.tile_pool(name="persist", bufs=1) as pp,
            tc.tile_pool(name="work", bufs=3) as wp,
            tc.tile_pool(name="gather", bufs=3) as gp,
            tc.tile_pool(name="msgp", bufs=2) as mp,
            tc.tile_pool(name="xt", bufs=2) as xtp,
            tc.tile_pool(name="xtw", bufs=1) as xwp,
            tc.tile_pool(name="pe_pool", bufs=3, space="PSUM") as pep,
            tc.tile_pool(name="pd_pool", bufs=2, space="PSUM") as pdp,
            tc.tile_pool(name="pt_pool", bufs=2, space="PSUM") as ptp,
            tc.tile_pool(name="yq_pool", bufs=1, space="PSUM") as yqp,
        ):
            # ---- constants / inputs -> SBUF
            ident_sb = pp.tile([128, 128], F32, tag="ident")
            nc.sync.dma_start(ident_sb[:], ident_c[:])
            ones_sb = pp.tile([128, 1], F32, tag="ones")
            nc.vector.memset(ones_sb[:], 1.0)

            xT0_sb = pp.tile([cfg.in_feat, cfg.npc_pad], BF16, tag="xT0")
            nc.sync.dma_start(xT0_sb[:], xT0_d[:])
            waug_sb, bias_sb = [], []
            for l in range(cfg.n_layers):
                fin, fo = cfg.f_in[l], cfg.f_out[l]
                p = min(fin, 128)
                kt = (fin + 127) // 128
                w = pp.tile([p, kt, fo + 16], BF16, tag=f"waug{l}")
                nc.sync.dma_start(w[:], waug_d[l].rearrange("(kt p) f -> p kt f", p=p))
                waug_sb.append(w)
                b = pp.tile([128, fo], F32, tag=f"bias{l}")
                nc.sync.dma_start(b[:], bias_d[l][:])
                bias_sb.append(b)
            gidx_sb = pp.tile([128, 8 * ttot], I16, tag="gidx")
            nc.sync.dma_start(gidx_sb[:], gidx_d[:])
            g_sb = pp.tile([128, 128 * nblk], F32, tag="gmat")
            nc.sync.dma_start(g_sb[:], g_d[:])
            ysel_sb = pp.tile([128, cfg.gpc], F32, tag="ysel")
            nc.sync.dma_start(ysel_sb[:], ysel_d[:])
            fcw_sb = pp.tile([128, nblk, 64], F32, tag="fcw")
            nc.sync.dma_start(fcw_sb[:], fcwn_d.rearrange("(b p) f -> p b f", p=128))
            p_sb = pp.tile([128, nblk], F32, tag="p_sb")

            dma_sems = tc.sems.swdge_block()
            prep_count = [0]

            state = {}  # layer -> (rowbf, dwin, sd32)
            xT_of = {}  # layer l -> xT tile holding x'(l)^T (input to l+1)

            def alloc_layer_state(l):
                rowbf = xtp.tile([128, nblk, cfg.rowp[l]], BF16, tag="rowbf")
                dwin = xtp.tile([128, nblk, 8], BF16, tag="dwin")
                sd32 = xtp.tile([128, nblk, 8], F32, tag="sd32")
                state[l] = (rowbf, dwin, sd32)

            def node_mm(l, b):
                """Node matmul + bf16 row build + h_in DMA for layer l, window b."""
                fin, fo = cfg.f_in[l], cfg.f_out[l]
                kt_in = (fin + 127) // 128
                split0 = fo + 16 > 512
                rowbf, dwin, sd32 = state[l]
                ph = pep.tile([128, 512], F32, tag="pe")
                if split0:
                    phs = pdp.tile([128, 96], F32, tag="pd")
                else:
                    phs = None
                for k in range(kt_in):
                    if l == 0:
                        lh = xT0_sb[:, b * 128 : (b + 1) * 128]
                    else:
                        lh = xT_of[l - 1][:, k, b * 128 : (b + 1) * 128]
                    if split0:
                        nc.tensor.matmul(
                            ph[:, 0:fo], lhsT=lh, rhs=waug_sb[l][:, k, 0:fo],
                            start=(k == 0), stop=(k == kt_in - 1),
                        )
                        nc.tensor.matmul(
                            phs[:, 68:84], lhsT=lh,
                            rhs=waug_sb[l][:, k, fo : fo + 16],
                            start=(k == 0), stop=(k == kt_in - 1),
                        )
                    else:
                        nc.tensor.matmul(
                            ph[:, 0 : fo + 16], lhsT=lh,
                            rhs=waug_sb[l][:, k, 0 : fo + 16],
                            start=(k == 0), stop=(k == kt_in - 1),
                        )
                sdp = phs[:, 68:84] if split0 else ph[:, fo : fo + 16]
                nc.scalar.copy(rowbf[:, b, 0:fo], ph[:, 0:fo])
                sdtmp = wp.tile([128, 16], F32, tag="sdtmp")
                nc.scalar.copy(sdtmp[:], sdp)
                nc.vector.tensor_tensor(
                    out=sd32[:, b, :], in0=sdtmp[:, 0:8], in1=sdtmp[:, 8:16],
                    op=mybir.AluOpType.add,
                )
                nc.vector.tensor_copy(rowbf[:, b, fo : fo + 4], sd32[:, b, 0:4])
                stmp = wp.tile([128, 4], F32, tag="stmp")
                nc.vector.tensor_copy(stmp[:], rowbf[:, b, fo : fo + 4])
                nc.vector.tensor_tensor(
                    out=rowbf[:, b, fo + 4 : fo + 8],
                    in0=sd32[:, b, 0:4], in1=stmp[:],
                    op=mybir.AluOpType.subtract,
                )
                nc.vector.tensor_copy(dwin[:, b, 0:4], sd32[:, b, 4:8])
                dtmp = wp.tile([128, 4], F32, tag="dtmp")
                nc.vector.tensor_copy(dtmp[:], dwin[:, b, 0:4])
                nc.vector.tensor_tensor(
                    out=dwin[:, b, 4:8],
                    in0=sd32[:, b, 4:8], in1=dtmp[:],
                    op=mybir.AluOpType.subtract,
                )
                if b < NWA:
                    dst = h_inA[l][b * 128 : (b + 1) * 128, :]
                else:
                    dst = h_inB[l][(b - NWA) * 128 : (b - NWA + 1) * 128, :]
                nc.sync.dma_start(dst, rowbf[:, b, :])

            def coll_A(l):
                nc.gpsimd.collective_compute(
                    AG, mybir.AluOpType.bypass, replica_groups=rg,
                    ins=[h_inA[l][:]],
                    outs=[h_glob[l][0 : cfg.n_cores * nwa, :]],
                )

            def coll_B(l):
                fo = cfg.f_out[l]
                rp = cfg.rowp[l]
                nc.gpsimd.collective_compute(
                    AG, mybir.AluOpType.bypass, replica_groups=rg,
                    ins=[h_inB[l][:]],
                    outs=[h_glob[l][cfg.n_cores * nwa : cfg.dummy, :]],
                )
                drow = wp.tile([1, rp], BF16, tag="drow")
                nc.vector.memset(drow[0:1, :], 0.0)
                nc.vector.memset(drow[0:1, fo : fo + 4], DUMMY_S)
                nc.sync.dma_start(h_glob[l][cfg.dummy : cfg.dummy + 1, :], drow[0:1, :])

            # ---- preamble: layer 0 node matmuls + split collectives
            alloc_layer_state(0)
            for b in range(nblk):
                node_mm(0, b)
                if b == NWA - 1:
                    coll_A(0)
            coll_B(0)

            for l in range(cfg.n_layers):
                fin, fo = cfg.f_in[l], cfg.f_out[l]
                rp = cfg.rowp[l]
                C = fo // HEADS
                split0 = fo + 16 > 512
                rowbf, dwin, sd32 = state[l]

                if l < cfg.n_layers - 1:
                    kt_out = (fo + 127) // 128
                    xT_next = xwp.tile(
                        [min(128, fo), kt_out, cfg.npc_pad], BF16, tag="xT"
                    )
                    xT_of[l] = xT_next
                else:
                    xT_next = None

                # ---- edge phase; node phase + next-layer node matmuls
                # pipelined one window behind
                hsrc_tiles = {}
                pe_tiles = {}
                pd_tiles = {}
                esf_tiles = {}
                st_tiles = {}
                ss_tiles = {}
                next_prep = 0
                pending = 0

                def emit_prep(g):
                    hs = gp.tile([128, T, rp], BF16, tag="hsrc")
                    sem = dma_sems[prep_count[0] % 8] if use_prep else None
                    prep_count[0] += 1
                    nc.gpsimd.dma_gather(
                        out_ap=hs[:],
                        in_ap=h_glob[l][:],
                        idxs_ap=gidx_sb[:, 8 * T * g : 8 * T * (g + 1)],
                        num_idxs=T * 128,
                        num_idxs_reg=T * 128,
                        elem_size=rp,
                        prepare_only=use_prep,
                        sem=sem,
                        single_packet=False,
                    )
                    hsrc_tiles[g] = hs

                def emit_onehot(g):
                    stw = wp.tile([128, T * 128], BF16, tag="stw")
                    nc.sync.dma_start(
                        stw[:], st_d[:, 128 * T * g : 128 * T * (g + 1)]
                    )
                    ssw = wp.tile([128, T * 128], BF16, tag="ssw")
                    nc.sync.dma_start(
                        ssw[:], ss_d[:, 128 * T * g : 128 * T * (g + 1)]
                    )
                    st_tiles[g] = stw
                    ss_tiles[g] = ssw

                def node_phase(g):
                    pe = pe_tiles.pop(g)
                    pdm = pd_tiles.pop(g)
                    esf = esf_tiles.pop(g)
                    den_ap = pdm[:, 64:68] if split0 else pe[:, fo : fo + 4]
                    den = wp.tile([128, 4], F32, tag="den")
                    nc.vector.tensor_tensor(
                        out=den[:], in0=den_ap, in1=esf[:],
                        op=mybir.AluOpType.add,
                    )
                    rec = wp.tile([128, 4], F32, tag="rec")
                    nc.vector.reciprocal(rec[:], den[:])
                    msum = wp.tile([128, fo], F32, tag="msum")
                    xp = wp.tile([128, fo], F32, tag="xp")
                    for h in range(HEADS):
                        sl = slice(h * C, (h + 1) * C)
                        nc.vector.scalar_tensor_tensor(
                            out=msum[:, sl],
                            in0=rowbf[:, g, h * C : (h + 1) * C],
                            scalar=esf[:, h : h + 1],
                            in1=pe[:, h * C : (h + 1) * C],
                            op0=mybir.AluOpType.mult,
                            op1=mybir.AluOpType.add,
                        )
                        nc.vector.scalar_tensor_tensor(
                            out=xp[:, sl],
                            in0=msum[:, sl],
                            scalar=rec[:, h : h + 1],
                            in1=bias_sb[l][:, sl],
                            op0=mybir.AluOpType.mult,
                            op1=mybir.AluOpType.add,
                        )
                    xm = wp.tile([128, fo], F32, tag="xm")
                    # (xp min 0) min xp == min(xp, 0); STT form avoids the
                    # pathologically slow TensorScalarPtr lowering
                    nc.vector.scalar_tensor_tensor(
                        out=xm[:], in0=xp[:], scalar=0.0, in1=xp[:],
                        op0=mybir.AluOpType.min, op1=mybir.AluOpType.min,
                    )
                    nc.scalar.activation(
                        out=xm[:], in_=xm[:], func=mybir.ActivationFunctionType.Exp
                    )
                    xn = wp.tile([128, fo], F32, tag="xn")
                    nc.vector.scalar_tensor_tensor(
                        out=xn[:], in0=xm[:], scalar=-1.0, in1=xp[:],
                        op0=mybir.AluOpType.add, op1=mybir.AluOpType.max,
                    )
                    if use_dbg and l == 0 and g == 0:
                        nc.sync.dma_start(dbg_x1[:], xn[:])
                    if xT_next is not None:
                        for fb in range((fo + 127) // 128):
                            w = min(128, fo - fb * 128)
                            pt = ptp.tile([128, 128], F32, tag="pt")
                            nc.tensor.transpose(
                                pt[0:w, :], xn[:, fb * 128 : fb * 128 + w],
                                ident_sb[:],
                            )
                            nc.scalar.copy(
                                xT_next[0:w, fb, g * 128 : (g + 1) * 128], pt[0:w, :]
                            )
                    else:
                        junk = wp.tile([128, 64], F32, tag="junk")
                        nc.vector.scalar_tensor_tensor(
                            out=junk[:], in0=xn[:, 0:64], scalar=1.0,
                            in1=fcw_sb[:, g, :],
                            op0=mybir.AluOpType.mult, op1=mybir.AluOpType.mult,
                            accum_out=p_sb[:, g : g + 1],
                        )

                def tail_work(g):
                    """Pipelined work for completed window g: node phase of
                    layer l + node matmul (and collectives) of layer l+1."""
                    node_phase(g)
                    if l + 1 < cfg.n_layers:
                        if g == 0:
                            alloc_layer_state(l + 1)
                        node_mm(l + 1, g)
                        if g == NWA - 1:
                            coll_A(l + 1)
                        elif g == nblk - 1:
                            coll_B(l + 1)

                emit_onehot(0)
                for g in range(nblk):
                    while next_prep <= min(g + AHEAD, nblk - 1):
                        emit_prep(next_prep)
                        next_prep += 1
                        pending += 1
                    if pending and use_prep:
                        nc.gpsimd.trigger_dma(count=None)
                    pending = 0
                    if g + 1 < nblk:
                        emit_onehot(g + 1)
                    hsrc = hsrc_tiles.pop(g)
                    stw = st_tiles.pop(g)
                    ssw = ss_tiles.pop(g)

                    # self-loop scores for window g (f32, local)
                    esf = wp.tile([128, 4], F32, tag="esf")
                    nc.vector.tensor_tensor(
                        out=esf[:], in0=sd32[:, g, 0:4], in1=sd32[:, g, 4:8],
                        op=mybir.AluOpType.add,
                    )
                    nc.vector.scalar_tensor_tensor(
                        out=esf[:], in0=esf[:], scalar=0.2, in1=esf[:],
                        op0=mybir.AluOpType.mult, op1=mybir.AluOpType.max,
                    )
                    nc.scalar.activation(
                        out=esf[:], in_=esf[:], func=mybir.ActivationFunctionType.Exp
                    )
                    esf_tiles[g] = esf

                    # d-expand: pd[:, t*8:t*8+8] = St_t^T @ dwin[g]
                    pe = pep.tile([128, 512], F32, tag="pe")
                    pe_tiles[g] = pe
                    pdm = pdp.tile([128, 96], F32, tag="pd")
                    pd_tiles[g] = pdm
                    pd = pdm[:, 0:64].rearrange("p (t e) -> p t e", t=T)
                    for t in range(T):
                        nc.tensor.matmul(
                            pd[:, t, :],
                            lhsT=stw[:, 128 * t : 128 * (t + 1)],
                            rhs=dwin[:, g, :],
                            start=True, stop=True,
                        )
                    # e = lrelu(s_hi+s_lo + d_hi+d_lo); ee = exp(e) -> bf16
                    et = wp.tile([128, T, 4], F32, tag="et")
                    nc.vector.tensor_tensor(
                        out=et[:],
                        in0=hsrc[:, :, fo : fo + 4],
                        in1=hsrc[:, :, fo + 4 : fo + 8],
                        op=mybir.AluOpType.add,
                    )
                    nc.vector.tensor_tensor(
                        out=et[:], in0=et[:], in1=pd[:, :, 0:4],
                        op=mybir.AluOpType.add,
                    )
                    nc.vector.tensor_tensor(
                        out=et[:], in0=et[:], in1=pd[:, :, 4:8],
                        op=mybir.AluOpType.add,
                    )
                    nc.vector.scalar_tensor_tensor(
                        out=et[:], in0=et[:], scalar=0.2, in1=et[:],
                        op0=mybir.AluOpType.mult, op1=mybir.AluOpType.max,
                    )
                    etb = wp.tile([128, T, 4], BF16, tag="etb")
                    nc.scalar.activation(
                        out=etb[:], in_=et[:], func=mybir.ActivationFunctionType.Exp
                    )
                    # ee pre-expanded per head on the Scalar engine so the DVE
                    # multiply below runs contiguous x contiguous at full rate
                    eex = mp.tile([128, T, fo], BF16, tag="eex")
                    et_b = bass.AP(et.tensor, et.offset, list(et.ap) + [[0, C]])
                    nc.scalar.activation(
                        out=eex[:], in_=et_b, func=mybir.ActivationFunctionType.Exp
                    )
                    if use_dbg and l == 0 and g == 0:
                        hsf = wp.tile([128, rp], F32, tag="hsf")
                        nc.vector.tensor_copy(hsf[:], hsrc[:, 0, :])
                        nc.sync.dma_start(dbg_hs[:], hsf[:])

                    # msg = ee * h (contiguous operands; DVE at full rate)
                    msg = mp.tile([128, T, fo + 4], BF16, tag="msg")
                    for t in range(T):
                        nc.vector.tensor_tensor(
                            out=msg[:, t, 0:fo],
                            in0=hsrc[:, t, 0:fo],
                            in1=eex[:, t, :],
                            op=mybir.AluOpType.mult,
                        )
                    nc.vector.tensor_copy(msg[:, :, fo : fo + 4], etb[:])
                    for t in range(T):
                        lhsT = ssw[:, 128 * t : 128 * (t + 1)]
                        if split0:
                            nc.tensor.matmul(
                                pe[:, 0:fo], lhsT=lhsT, rhs=msg[:, t, 0:fo],
                                start=(t == 0), stop=(t == T - 1),
                            )
                            nc.tensor.matmul(
                                pdm[:, 64:68], lhsT=lhsT,
                                rhs=msg[:, t, fo : fo + 4],
                                start=(t == 0), stop=(t == T - 1),
                            )
                        else:
                            nc.tensor.matmul(
                                pe[:, 0 : fo + 4], lhsT=lhsT, rhs=msg[:, t, :],
                                start=(t == 0), stop=(t == T - 1),
                            )
                    if g > 0:
                        tail_work(g - 1)
                tail_work(nblk - 1)

            # ---- readout: per-graph partial sums via one-hot G matmul
            if use_dbg:
                nc.sync.dma_start(dbg_p[:], p_sb[:])
            yp = yqp.tile([128, 1], F32, tag="yq")
            for g in range(nblk):
                nc.tensor.matmul(
                    yp[:],
                    lhsT=g_sb[:, 128 * g : 128 * (g + 1)],
                    rhs=p_sb[:, g : g + 1],
                    start=(g == 0), stop=(g == nblk - 1),
                )
            ypart_sb = pp.tile([128, 1], F32, tag="ypart")
            nc.scalar.copy(ypart_sb[:], yp[:])
            nc.sync.dma_start(ypart_d[:], ypart_sb[:])
            nc.gpsimd.collective_compute(
                AG,
                mybir.AluOpType.bypass,
                replica_groups=rg,
                ins=[ypart_d[:]],
                outs=[yglob_d[:]],
            )
            ya = pp.tile([128, cfg.n_cores], F32, tag="ya")
            nc.sync.dma_start(
                ya[:], yglob_d.rearrange("(c g) one -> g (c one)", g=128)
            )
            if use_dbg:
                nc.sync.dma_start(dbg_ya[:], ya[:])
            ysum = pp.tile([128, 1], F32, tag="ysum")
            yjunk = pp.tile([128, cfg.n_cores], F32, tag="yjunk")
            nc.vector.scalar_tensor_tensor(
                out=yjunk[:], in0=ya[:], scalar=1.0,
                in1=ones_sb[:, 0:1].to_broadcast([128, cfg.n_cores]),
                op0=mybir.AluOpType.mult, op1=mybir.AluOpType.mult,
                accum_out=ysum[:],
            )
            yq = yqp.tile([cfg.gpc, 1], F32, tag="yq")
            nc.tensor.matmul(
                yq[:], lhsT=ysel_sb[:], rhs=ysum[:], start=True, stop=True
            )
            y_sb = pp.tile([cfg.gpc, 1], F32, tag="y_sb")
            nc.scalar.copy(y_sb[:], yq[:])
            nc.sync.dma_start(y_d[:], y_sb[:])

    nc.compile()
    return nc


# ------------------------------------------------------------------- driver

last_results = None  # BassKernelResults of the most recent run (for test.py)
_cache = {}


def _prepare(cfg, inputs):
    T, perm_pad, per_core = preprocess(cfg, np.asarray(inputs["edge_index"]))
    x = np.asarray(inputs["x"], dtype=np.float32)
    fcw = np.asarray(inputs["fcw"], dtype=np.float32)
    fcb = float(np.asarray(inputs["fcb"]).reshape(-1)[0])
    waugs = [
        make_waug(
            np.asarray(inputs[f"W{l + 1}"], np.float32),
            np.asarray(inputs[f"as{l + 1}"], np.float32),
            np.asarray(inputs[f"ad{l + 1}"], np.float32),
        )
        for l in range(cfg.n_layers)
    ]
    biases = [
        np.tile(np.asarray(inputs[f"b{l + 1}"], np.float32)[None, :], (128, 1))
        for l in range(cfg.n_layers)
    ]
    n = cfg.n_nodes
    nodes = np.arange(n)
    fcw_node_full = fcw.reshape(cfg.npg, 64)[nodes % cfg.npg]  # [N, 64]
    graph_of = nodes // cfg.npg

    in_maps = []
    for c in range(cfg.n_cores):
        # nodes mapped to this core, by padded-local position
        loc = perm_pad[nodes] - c * cfg.npc_pad
        m = (loc >= 0) & (loc < cfg.npc_pad)
        vsel = nodes[m]
        lsel = loc[m]
        xT0 = np.zeros((cfg.in_feat, cfg.npc_pad), np.float32)
        xT0[:, lsel] = x[vsel].T
        fcwn = np.zeros((cfg.npc_pad, 64), np.float32)
        fcwn[lsel] = fcw_node_full[vsel]
        gmat = np.zeros((128, 128 * cfg.nblk), np.float32)
        for v, lo in zip(vsel, lsel):
            w, s = lo // 128, lo % 128
            gmat[s, w * 128 + graph_of[v]] = 1.0
        ysel = np.zeros((128, cfg.gpc), np.float32)
        for g in range(cfg.gpc):
            ysel[c * cfg.gpc + g, g] = 1.0
        m_ = dict(
            xT0=np.ascontiguousarray(xT0.astype(ml_dtypes.bfloat16)),
            gidx=per_core[c]["gidx"],
            st=per_core[c]["st"],
            ss=per_core[c]["s"],
            gmat=np.ascontiguousarray(gmat),
            ysel=np.ascontiguousarray(ysel),
            fcwn=np.ascontiguousarray(fcwn),
        )
        for l in range(cfg.n_layers):
            m_[f"waug{l}"] = waugs[l]
            m_[f"bias{l}"] = biases[l]
        in_maps.append(m_)
    return T, in_maps, fcb


def _ensure_ntff_hook():
    """Shim antenv.axon_hooks (absent in this image) so BASS_TRACE works."""
    try:
        from antenv.axon_hooks import get_axon_ntff_profile_hook  # noqa: F401

        return
    except ImportError:
        pass
    try:
        import types

        import antenv

        mod = types.ModuleType("antenv.axon_hooks")
        holder = [None]
        mod.set_axon_ntff_profile_hook = lambda h: holder.__setitem__(0, h)
        mod.get_axon_ntff_profile_hook = lambda: holder[0]
        sys.modules["antenv.axon_hooks"] = mod
        antenv.axon_hooks = mod
        from trn_agent_boot.trn_boot import _ntff_profile_via_ctypes

        h = _ntff_profile_via_ctypes("/opt/axon/libaxon_pjrt.so")
        if h is not None:
            holder[0] = h
    except Exception:
        pass


def run(cfg, inputs, trace=False, dbg=False):
    global last_results
    if trace or os.environ.get("BASS_TRACE"):
        _ensure_ntff_hook()
    T, in_maps, fcb = _prepare(cfg, inputs)
    key = (cfg.n_nodes, T, dbg)
    if key not in _cache:
        _cache[key] = build_kernel(cfg, T, dbg=dbg)
    nc = _cache[key]
    res = run_bass_kernel_spmd(
        nc, in_maps, core_ids=list(range(cfg.n_cores)), trace=trace
    )
    last_results = res
    y = np.concatenate([r["y"].reshape(-1) for r in res.results])
    return (y.reshape(-1, 1) + fcb).astype(np.float32)


def kernel(**inputs) -> np.ndarray:
    cfg = default_cfg()
    return run(cfg, inputs)


# revision 36
# speedup vs baseline: 1.1086x; 1.1086x over previous
"""Trainium2 Bass kernel for nn_GATsimple (4-layer GAT + graph readout).

Self-contained: takes the FULL inputs from setup_inputs(), shards across 8
NeuronCores internally (data-parallel over dst-node ranges after a host-side
degree-balanced node->core remap), runs a Bass/Tile kernel via
run_bass_kernel_spmd, and returns the FULL [128, 1] output.

v2 design (vs v1):
  - bf16 h table: row = [h (fo) | s_hi (4) | s_lo (4) | pad] (256B-multiple
    rows); scores moved exactly via hi/lo bf16 pairs.
  - one-hot St (d-expand) and S (segment-sum) matrices are static per edge
    instance -> precomputed on host, loaded to SBUF once, reused all layers.
  - self-loops removed from the edge list and folded into the node phase
    (local, exact f32).
  - host remap balances in-degree across (core, window) bins -> uniform
    T tiles/window everywhere.
  - all matmuls bf16 (x, waug, msg, one-hots); PSUM f32.
  - gathers via dma_gather prepare_only + trigger_dma, prepped ahead.
  - node phase software-pipelined one window behind the edge phase.

Per layer:
  A. node matmul h_aug[v] = x[v] @ [W | WA_hi | WA_lo]  (A = blockdiag of
     a_src/a_dst); build bf16 table rows + dwin (d hi/lo) + sd32 (f32 s,d).
  B. AllGather table -> h_glob; write dummy row (s = -100).
  C. per 128-dst-node window: trigger prepped gather of h[src] rows;
     d-expand via St matmul; e = exp(lrelu(s+d)) (f32, out bf16);
     msg = ee*h (bf16); segment-sum via S matmul accumulated in PSUM.
  D. node phase (one window delayed): add self-loop term, xp = msg/denom
     + bias, ELU, PE-transpose into xT (bf16) for the next layer; last
     layer: p = x4 . fcw_node, graph accumulation via one-hot G matmul.
Readout: per-core partial per-graph sums [128,1] -> AllGather -> reduce ->
per-core one-hot column select -> y slice [16,1]; driver adds fcb.
"""

import os
import sys

import ml_dtypes
import numpy as np

for _p in ("/opt/trn_rl_repo", "/root/.axon_site/_ro/trn_rl_repo"):
    if os.path.isdir(_p) and _p not in sys.path:
        sys.path.append(_p)

import concourse.bass as bass
import concourse.bacc as bacc
import concourse.mybir as mybir
import concourse.tile as tile
from concourse.bass_utils import run_bass_kernel_spmd

F32 = mybir.dt.float32
BF16 = mybir.dt.bfloat16
I16 = mybir.dt.int16

N_CORES = 8
HEADS = 4
NWA = 9  # windows per core in AllGather chunk A (rest go to chunk B)
DUMMY_S = -100.0  # dummy-source s_hi; exp(lrelu(-100+d)) ~ 2e-9


class Cfg:
    def __init__(self, n_nodes, npg, in_feat, layer_out, n_cores=N_CORES):
        assert n_nodes % n_cores == 0
        self.n_nodes = n_nodes
        self.npg = npg
        self.n_graphs = n_nodes // npg
        self.n_cores = n_cores
        self.npc = n_nodes // n_cores
        self.nblk = (self.npc + 127) // 128
        self.npc_pad = self.nblk * 128
        self.dummy = n_cores * self.npc_pad
        self.nrows = self.dummy + 1
        self.in_feat = in_feat
        self.layer_out = layer_out
        self.f_out = [HEADS * c for c in layer_out]
        self.f_in = [in_feat] + self.f_out[:-1]
        self.n_layers = len(layer_out)
        self.gpc = self.n_graphs // n_cores
        # bf16 table rows: fo + 8 useful, padded to a multiple of 128 elems
        self.rowp = [max(128, ((fo + 8 + 127) // 128) * 128) for fo in self.f_out]


def default_cfg():
    return Cfg(n_nodes=17024, npg=133, in_feat=64, layer_out=[128, 64, 32, 16])


# ------------------------------------------------------------ host preprocess


def balance_nodes(cfg, dst):
    """Assign nodes to (core, window) bins, balancing in-degree.

    Returns (assign[node] -> bin, T tiles/window). Bin b = core*nblk + win.
    Window nblk-1 of each core holds npc - (nblk-1)*128 nodes; others 128.
    """
    import heapq

    n = cfg.n_nodes
    deg = np.bincount(dst, minlength=n).astype(np.int64)
    nbins = cfg.n_cores * cfg.nblk
    cap = np.full(nbins, 128, dtype=np.int64)
    cap[cfg.nblk - 1 :: cfg.nblk] = cfg.npc - (cfg.nblk - 1) * 128
    assert cap.sum() == n
    order = np.argsort(-deg, kind="stable")
    load = np.zeros(nbins, dtype=np.int64)
    cnt = np.zeros(nbins, dtype=np.int64)
    assign = np.empty(n, dtype=np.int64)
    heap = [(0, b) for b in range(nbins)]
    heapq.heapify(heap)
    for v in order:
        while True:
            l, b = heapq.heappop(heap)
            if cnt[b] < cap[b]:
                break
            # bin full; drop it
        assign[v] = b
        cnt[b] += 1
        load[b] += deg[v]
        if cnt[b] < cap[b]:
            heapq.heappush(heap, (load[b], b))
    T = max(1, int(-(-load.max() // 128)))
    return assign, T


def preprocess(cfg, edge_index):
    """Host preprocessing: node remap + per-core edge buckets + one-hots.

    Returns (T, perm_pad[node]->padded_id, per_core list of dicts).
    """
    src = np.asarray(edge_index[0], dtype=np.int64)
    dst = np.asarray(edge_index[1], dtype=np.int64)
    assign, T = balance_nodes(cfg, dst)

    # slot assignment within each bin, in node order
    nbins = cfg.n_cores * cfg.nblk
    slot = np.empty(cfg.n_nodes, dtype=np.int64)
    fill = np.zeros(nbins, dtype=np.int64)
    for v in range(cfg.n_nodes):
        b = assign[v]
        slot[v] = fill[b]
        fill[b] += 1
    core = assign // cfg.nblk
    win = assign % cfg.nblk
    perm_pad = core * cfg.npc_pad + win * 128 + slot  # node -> padded id

    # table row id in the A/B-half AllGather layout: windows 0..NWA-1 of
    # each core land in half A (core-major blocks of NWA*128 rows), the rest
    # in half B (core-major blocks at offset n_cores*NWA*128)
    nwa = NWA * 128
    nwb = cfg.npc_pad - nwa
    local = win * 128 + slot
    tab_row = np.where(
        local < nwa,
        core * nwa + local,
        cfg.n_cores * nwa + core * nwb + (local - nwa),
    )
    src_tab = tab_row[src]
    dst_core = core[dst]
    dst_win = win[dst]
    dst_slot = slot[dst]

    per_core = []
    for c in range(cfg.n_cores):
        m = dst_core == c
        e_src = src_tab[m]
        e_win = dst_win[m]
        e_slot = dst_slot[m]
        order = np.argsort(e_win, kind="stable")
        e_src, e_win, e_slot = e_src[order], e_win[order], e_slot[order]
        counts = np.bincount(e_win, minlength=cfg.nblk)
        assert counts.max() <= T * 128, (counts.max(), T * 128)
        starts = np.concatenate([[0], np.cumsum(counts)])
        tot = T * 128
        gidx_cols, st_cols, s_cols = [], [], []
        for g in range(cfg.nblk):
            s0, s1 = starts[g], starts[g + 1]
            cnt = s1 - s0
            sp = np.full(tot, cfg.dummy, dtype=np.int64)
            dl = np.zeros(tot, dtype=np.int64)
            sp[:cnt] = e_src[s0:s1]
            dl[:cnt] = e_slot[s0:s1]
            wrap = sp.astype(np.int16).reshape(-1, 16).T  # [16, T*8]
            gidx_cols.append(np.tile(wrap, (8, 1)))  # [128, T*8]
            # one-hots per tile: St[p, t*128+e] = (dl[t*128+e] == p)
            dlt = dl.reshape(T, 128)
            oh = (dlt[:, None, :] == np.arange(128)[None, :, None])  # [T,128p,128e]
            st_cols.append(
                np.concatenate(list(oh), axis=1).astype(ml_dtypes.bfloat16)
            )  # [128, T*128]
            s_cols.append(
                np.concatenate(
                    [oh[t].T for t in range(T)], axis=1
                ).astype(ml_dtypes.bfloat16)
            )  # [128e-part, T*128n]
        per_core.append(
            dict(
                gidx=np.ascontiguousarray(np.concatenate(gidx_cols, axis=1)),
                st=np.ascontiguousarray(np.concatenate(st_cols, axis=1)),
                s=np.ascontiguousarray(np.concatenate(s_cols, axis=1)),
            )
        )
    return T, perm_pad, per_core


def make_waug(W, a_s, a_d):
    """[W | (W@A)_hi | (W@A)_lo] as bf16; A = blockdiag score vectors."""
    fin, fout = W.shape
    H, C = a_s.shape
    assert H * C == fout
    A = np.zeros((fout, 2 * H), dtype=np.float64)
    for h in range(H):
        A[h * C : (h + 1) * C, h] = a_s[h]
        A[h * C : (h + 1) * C, H + h] = a_d[h]
    wa = W.astype(np.float64) @ A  # [fin, 8]
    wa_hi = wa.astype(ml_dtypes.bfloat16)
    wa_lo = (wa - wa_hi.astype(np.float64)).astype(ml_dtypes.bfloat16)
    return np.ascontiguousarray(
        np.concatenate(
            [W.astype(ml_dtypes.bfloat16), wa_hi, wa_lo], axis=1
        )
    )  # [fin, fout+16] bf16


# ---------------------------------------------------------------- bass kernel


def build_kernel(cfg, T, dbg=False):
    nblk = cfg.nblk
    ttot = T * nblk
    nwa = NWA * 128                 # rows per core in chunk A
    nwb = cfg.npc_pad - nwa         # rows per core in chunk B
    nc = bacc.Bacc(
        "TRN2",
        target_bir_lowering=False,
        debug=False,
        num_devices=cfg.n_cores,
    )

    # ---- I/O declarations (all per-core)
    xT0_d = nc.dram_tensor("xT0", [cfg.in_feat, cfg.npc_pad], BF16, kind="ExternalInput")
    waug_d, bias_d = [], []
    for l in range(cfg.n_layers):
        waug_d.append(
            nc.dram_tensor(
                f"waug{l}", [cfg.f_in[l], cfg.f_out[l] + 16], BF16, kind="ExternalInput"
            )
        )
        bias_d.append(
            nc.dram_tensor(f"bias{l}", [128, cfg.f_out[l]], F32, kind="ExternalInput")
        )
    gidx_d = nc.dram_tensor("gidx", [128, 8 * ttot], I16, kind="ExternalInput")
    st_d = nc.dram_tensor("st", [128, 128 * ttot], BF16, kind="ExternalInput")
    ss_d = nc.dram_tensor("ss", [128, 128 * ttot], BF16, kind="ExternalInput")
    g_d = nc.dram_tensor("gmat", [128, 128 * nblk], F32, kind="ExternalInput")
    ysel_d = nc.dram_tensor("ysel", [128, cfg.gpc], F32, kind="ExternalInput")
    fcwn_d = nc.dram_tensor("fcwn", [cfg.npc_pad, 64], F32, kind="ExternalInput")
    y_d = nc.dram_tensor("y", [cfg.gpc, 1], F32, kind="ExternalOutput")

    # internal DRAM; table layout: [A: core-major nwa blocks | B: core-major
    # nwb blocks | dummy row]
    h_inA, h_inB, h_glob = [], [], []
    for l in range(cfg.n_layers):
        rp = cfg.rowp[l]
        h_inA.append(nc.dram_tensor(f"h_inA{l}", [nwa, rp], BF16))
        h_inB.append(nc.dram_tensor(f"h_inB{l}", [nwb, rp], BF16))
        h_glob.append(
            nc.dram_tensor(f"h_glob{l}", [cfg.nrows, rp], BF16, addr_space="Shared")
        )
    ypart_d = nc.dram_tensor("ypart", [128, 1], F32)
    yglob_d = nc.dram_tensor("yglob", [cfg.n_cores * 128, 1], F32, addr_space="Shared")
    use_dbg = os.environ.get("GAT_DBG", "0") == "1"
    if use_dbg:
        dbg_p = nc.dram_tensor("dbg_p", [128, cfg.nblk], F32, kind="ExternalOutput")
        dbg_ya = nc.dram_tensor("dbg_ya", [128, cfg.n_cores], F32, kind="ExternalOutput")
        dbg_hs = nc.dram_tensor("dbg_hs", [128, cfg.rowp[0]], F32, kind="ExternalOutput")
        dbg_x1 = nc.dram_tensor("dbg_x1", [128, cfg.f_out[0]], F32, kind="ExternalOutput")

    ident_c = nc.inline_tensor(np.eye(128, dtype=np.float32), name="ident_c")

    rg = [list(range(cfg.n_cores))]
    AG = "AllGather"
    use_prep = os.environ.get("GAT_PREP", "0") == "1"
    AHEAD = int(os.environ.get("GAT_AHEAD", "2"))

    with tile.TileContext(nc) as tc:
        with (
            tc.tile_pool(name="persist", bufs=1) as pp,
            tc.tile_pool(name="work", bufs=3) as wp,
            tc.tile_pool(name="gather", bufs=4) as gp,
            tc.tile_pool(name="msgp", bufs=2) as mp,
            tc.tile_pool(name="xt", bufs=2) as xtp,
            tc.tile_pool(name="xtw", bufs=1) as xwp,
            tc.tile_pool(name="pe_pool", bufs=3, space="PSUM") as pep,
            tc.tile_pool(name="pd_pool", bufs=2, space="PSUM") as pdp,
            tc.tile_pool(name="pt_pool", bufs=2, space="PSUM") as ptp,
            tc.tile_pool(name="yq_pool", bufs=1, space="PSUM") as yqp,
        ):
            # ---- constants / inputs -> SBUF
            ident_sb = pp.tile([128, 128], F32, tag="ident")
            nc.sync.dma_start(ident_sb[:], ident_c[:])
            ones_sb = pp.tile([128, 1], F32, tag="ones")
            nc.vector.memset(ones_sb[:], 1.0)

            xT0_sb = pp.tile([cfg.in_feat, cfg.npc_pad], BF16, tag="xT0")
            nc.sync.dma_start(xT0_sb[:], xT0_d[:])
            waug_sb, bias_sb = [], []
            for l in range(cfg.n_layers):
                fin, fo = cfg.f_in[l], cfg.f_out[l]
                p = min(fin, 128)
                kt = (fin + 127) // 128
                w = pp.tile([p, kt, fo + 16], BF16, tag=f"waug{l}")
                nc.sync.dma_start(w[:], waug_d[l].rearrange("(kt p) f -> p kt f", p=p))
                waug_sb.append(w)
                b = pp.tile([128, fo], F32, tag=f"bias{l}")
                nc.sync.dma_start(b[:], bias_d[l][:])
                bias_sb.append(b)
            gidx_sb = pp.tile([128, 8 * ttot], I16, tag="gidx")
            nc.sync.dma_start(gidx_sb[:], gidx_d[:])
            g_sb = pp.tile([128, 128 * nblk], F32, tag="gmat")
            nc.sync.dma_start(g_sb[:], g_d[:])
            ysel_sb = pp.tile([128, cfg.gpc], F32, tag="ysel")
            nc.sync.dma_start(ysel_sb[:], ysel_d[:])
            fcw_sb = pp.tile([128, nblk, 64], F32, tag="fcw")
            nc.sync.dma_start(fcw_sb[:], fcwn_d.rearrange("(b p) f -> p b f", p=128))
            p_sb = pp.tile([128, nblk], F32, tag="p_sb")

            dma_sems = tc.sems.swdge_block()
            prep_count = [0]

            state = {}  # layer -> (rowbf, dwin, sd32)
            xT_of = {}  # layer l -> xT tile holding x'(l)^T (input to l+1)

            def alloc_layer_state(l):
                rowbf = xtp.tile([128, nblk, cfg.rowp[l]], BF16, tag="rowbf")
                dwin = xtp.tile([128, nblk, 8], BF16, tag="dwin")
                sd32 = xtp.tile([128, nblk, 8], F32, tag="sd32")
                state[l] = (rowbf, dwin, sd32)

            def node_mm(l, b):
                """Node matmul + bf16 row build + h_in DMA for layer l, window b."""
                fin, fo = cfg.f_in[l], cfg.f_out[l]
                kt_in = (fin + 127) // 128
                split0 = fo + 16 > 512
                rowbf, dwin, sd32 = state[l]
                ph = pep.tile([128, 512], F32, tag="pe")
                if split0:
                    phs = pdp.tile([128, 96], F32, tag="pd")
                else:
                    phs = None
                for k in range(kt_in):
                    if l == 0:
                        lh = xT0_sb[:, b * 128 : (b + 1) * 128]
                    else:
                        lh = xT_of[l - 1][:, k, b * 128 : (b + 1) * 128]
                    if split0:
                        nc.tensor.matmul(
                            ph[:, 0:fo], lhsT=lh, rhs=waug_sb[l][:, k, 0:fo],
                            start=(k == 0), stop=(k == kt_in - 1),
                        )
                        nc.tensor.matmul(
                            phs[:, 68:84], lhsT=lh,
                            rhs=waug_sb[l][:, k, fo : fo + 16],
                            start=(k == 0), stop=(k == kt_in - 1),
                        )
                    else:
                        nc.tensor.matmul(
                            ph[:, 0 : fo + 16], lhsT=lh,
                            rhs=waug_sb[l][:, k, 0 : fo + 16],
                            start=(k == 0), stop=(k == kt_in - 1),
                        )
                sdp = phs[:, 68:84] if split0 else ph[:, fo : fo + 16]
                nc.scalar.copy(rowbf[:, b, 0:fo], ph[:, 0:fo])
                sdtmp = wp.tile([128, 16], F32, tag="sdtmp")
                nc.scalar.copy(sdtmp[:], sdp)
                nc.vector.tensor_tensor(
                    out=sd32[:, b, :], in0=sdtmp[:, 0:8], in1=sdtmp[:, 8:16],
                    op=mybir.AluOpType.add,
                )
                nc.vector.tensor_copy(rowbf[:, b, fo : fo + 4], sd32[:, b, 0:4])
                stmp = wp.tile([128, 4], F32, tag="stmp")
                nc.vector.tensor_copy(stmp[:], rowbf[:, b, fo : fo + 4])
                nc.vector.tensor_tensor(
                    out=rowbf[:, b, fo + 4 : fo + 8],
                    in0=sd32[:, b, 0:4], in1=stmp[:],
                    op=mybir.AluOpType.subtract,
                )
                nc.vector.tensor_copy(dwin[:, b, 0:4], sd32[:, b, 4:8])
                dtmp = wp.tile([128, 4], F32, tag="dtmp")
                nc.vector.tensor_copy(dtmp[:], dwin[:, b, 0:4])
                nc.vector.tensor_tensor(
                    out=dwin[:, b, 4:8],
                    in0=sd32[:, b, 4:8], in1=dtmp[:],
                    op=mybir.AluOpType.subtract,
                )
                if b < NWA:
                    dst = h_inA[l][b * 128 : (b + 1) * 128, :]
                else:
                    dst = h_inB[l][(b - NWA) * 128 : (b - NWA + 1) * 128, :]
                nc.sync.dma_start(dst, rowbf[:, b, :])

            def coll_A(l):
                nc.gpsimd.collective_compute(
                    AG, mybir.AluOpType.bypass, replica_groups=rg,
                    ins=[h_inA[l][:]],
                    outs=[h_glob[l][0 : cfg.n_cores * nwa, :]],
                )

            def coll_B(l):
                fo = cfg.f_out[l]
                rp = cfg.rowp[l]
                nc.gpsimd.collective_compute(
                    AG, mybir.AluOpType.bypass, replica_groups=rg,
                    ins=[h_inB[l][:]],
                    outs=[h_glob[l][cfg.n_cores * nwa : cfg.dummy, :]],
                )
                drow = wp.tile([1, rp], BF16, tag="drow")
                nc.vector.memset(drow[0:1, :], 0.0)
                nc.vector.memset(drow[0:1, fo : fo + 4], DUMMY_S)
                nc.sync.dma_start(h_glob[l][cfg.dummy : cfg.dummy + 1, :], drow[0:1, :])

            # ---- preamble: layer 0 node matmuls + split collectives
            alloc_layer_state(0)
            for b in range(nblk):
                node_mm(0, b)
                if b == NWA - 1:
                    coll_A(0)
            coll_B(0)

            for l in range(cfg.n_layers):
                fin, fo = cfg.f_in[l], cfg.f_out[l]
                rp = cfg.rowp[l]
                C = fo // HEADS
                split0 = fo + 16 > 512
                rowbf, dwin, sd32 = state[l]

                if l < cfg.n_layers - 1:
                    kt_out = (fo + 127) // 128
                    xT_next = xwp.tile(
                        [min(128, fo), kt_out, cfg.npc_pad], BF16, tag="xT"
                    )
                    xT_of[l] = xT_next
                else:
                    xT_next = None

                # ---- edge phase; node phase + next-layer node matmuls
                # pipelined one window behind
                hsrc_tiles = {}
                pe_tiles = {}
                pd_tiles = {}
                esf_tiles = {}
                st_tiles = {}
                ss_tiles = {}
                next_prep = 0
                pending = 0

                def emit_prep(g):
                    hs = gp.tile([128, T, rp], BF16, tag="hsrc")
                    sem = dma_sems[prep_count[0] % 8] if use_prep else None
                    prep_count[0] += 1
                    nc.gpsimd.dma_gather(
                        out_ap=hs[:],
                        in_ap=h_glob[l][:],
                        idxs_ap=gidx_sb[:, 8 * T * g : 8 * T * (g + 1)],
                        num_idxs=T * 128,
                        num_idxs_reg=T * 128,
                        elem_size=rp,
                        prepare_only=use_prep,
                        sem=sem,
                        single_packet=False,
                    )
                    hsrc_tiles[g] = hs

                def emit_onehot(g):
                    stw = wp.tile([128, T * 128], BF16, tag="stw")
                    nc.sync.dma_start(
                        stw[:], st_d[:, 128 * T * g : 128 * T * (g + 1)]
                    )
                    ssw = wp.tile([128, T * 128], BF16, tag="ssw")
                    nc.sync.dma_start(
                        ssw[:], ss_d[:, 128 * T * g : 128 * T * (g + 1)]
                    )
                    st_tiles[g] = stw
                    ss_tiles[g] = ssw

                def node_phase(g):
                    pe = pe_tiles.pop(g)
                    pdm = pd_tiles.pop(g)
                    esf = esf_tiles.pop(g)
                    den_ap = pdm[:, 64:68] if split0 else pe[:, fo : fo + 4]
                    den = wp.tile([128, 4], F32, tag="den")
                    nc.vector.tensor_tensor(
                        out=den[:], in0=den_ap, in1=esf[:],
                        op=mybir.AluOpType.add,
                    )
                    rec = wp.tile([128, 4], F32, tag="rec")
                    nc.vector.reciprocal(rec[:], den[:])
                    msum = wp.tile([128, fo], F32, tag="msum")
                    xp = wp.tile([128, fo], F32, tag="xp")
                    for h in range(HEADS):
                        sl = slice(h * C, (h + 1) * C)
                        nc.vector.scalar_tensor_tensor(
                            out=msum[:, sl],
                            in0=rowbf[:, g, h * C : (h + 1) * C],
                            scalar=esf[:, h : h + 1],
                            in1=pe[:, h * C : (h + 1) * C],
                            op0=mybir.AluOpType.mult,
                            op1=mybir.AluOpType.add,
                        )
                        nc.vector.scalar_tensor_tensor(
                            out=xp[:, sl],
                            in0=msum[:, sl],
                            scalar=rec[:, h : h + 1],
                            in1=bias_sb[l][:, sl],
                            op0=mybir.AluOpType.mult,
                            op1=mybir.AluOpType.add,
                        )
                    xm = wp.tile([128, fo], F32, tag="xm")
                    # (xp min 0) min xp == min(xp, 0); STT form avoids the
                    # pathologically slow TensorScalarPtr lowering
                    nc.vector.scalar_tensor_tensor(
                        out=xm[:], in0=xp[:], scalar=0.0, in1=xp[:],
                        op0=mybir.AluOpType.min, op1=mybir.AluOpType.min,
                    )
                    nc.scalar.activation(
                        out=xm[:], in_=xm[:], func=mybir.ActivationFunctionType.Exp
                    )
                    xn = wp.tile([128, fo], F32, tag="xn")
                    nc.vector.scalar_tensor_tensor(
                        out=xn[:], in0=xm[:], scalar=-1.0, in1=xp[:],
                        op0=mybir.AluOpType.add, op1=mybir.AluOpType.max,
                    )
                    if use_dbg and l == 0 and g == 0:
                        nc.sync.dma_start(dbg_x1[:], xn[:])
                    if xT_next is not None:
                        for fb in range((fo + 127) // 128):
                            w = min(128, fo - fb * 128)
                            pt = ptp.tile([128, 128], F32, tag="pt")
                            nc.tensor.transpose(
                                pt[0:w, :], xn[:, fb * 128 : fb * 128 + w],
                                ident_sb[:],
                            )
                            nc.scalar.copy(
                                xT_next[0:w, fb, g * 128 : (g + 1) * 128], pt[0:w, :]
                            )
                    else:
                        junk = wp.tile([128, 64], F32, tag="junk")
                        nc.vector.scalar_tensor_tensor(
                            out=junk[:], in0=xn[:, 0:64], scalar=1.0,
                            in1=fcw_sb[:, g, :],
                            op0=mybir.AluOpType.mult, op1=mybir.AluOpType.mult,
                            accum_out=p_sb[:, g : g + 1],
                        )

                def tail_work(g):
                    """Pipelined work for completed window g: node phase of
                    layer l + node matmul (and collectives) of layer l+1."""
                    node_phase(g)
                    if l + 1 < cfg.n_layers:
                        if g == 0:
                            alloc_layer_state(l + 1)
                        node_mm(l + 1, g)
                        if g == NWA - 1:
                            coll_A(l + 1)
                        elif g == nblk - 1:
                            coll_B(l + 1)

                emit_onehot(0)
                for g in range(nblk):
                    while next_prep <= min(g + AHEAD, nblk - 1):
                        emit_prep(next_prep)
                        next_prep += 1
                        pending += 1
                    if pending and use_prep:
                        nc.gpsimd.trigger_dma(count=None)
                    pending = 0
                    if g + 1 < nblk:
                        emit_onehot(g + 1)
                    hsrc = hsrc_tiles.pop(g)
                    stw = st_tiles.pop(g)
                    ssw = ss_tiles.pop(g)

                    # self-loop scores for window g (f32, local)
                    esf = wp.tile([128, 4], F32, tag="esf")
                    nc.vector.tensor_tensor(
                        out=esf[:], in0=sd32[:, g, 0:4], in1=sd32[:, g, 4:8],
                        op=mybir.AluOpType.add,
                    )
                    nc.vector.scalar_tensor_tensor(
                        out=esf[:], in0=esf[:], scalar=0.2, in1=esf[:],
                        op0=mybir.AluOpType.mult, op1=mybir.AluOpType.max,
                    )
                    nc.scalar.activation(
                        out=esf[:], in_=esf[:], func=mybir.ActivationFunctionType.Exp
                    )
                    esf_tiles[g] = esf

                    # d-expand: pd[:, t*8:t*8+8] = St_t^T @ dwin[g]
                    pe = pep.tile([128, 512], F32, tag="pe")
                    pe_tiles[g] = pe
                    pdm = pdp.tile([128, 96], F32, tag="pd")
                    pd_tiles[g] = pdm
                    pd = pdm[:, 0:64].rearrange("p (t e) -> p t e", t=T)
                    for t in range(T):
                        nc.tensor.matmul(
                            pd[:, t, :],
                            lhsT=stw[:, 128 * t : 128 * (t + 1)],
                            rhs=dwin[:, g, :],
                            start=True, stop=True,
                        )
                    # e = lrelu(s_hi+s_lo + d_hi+d_lo); ee = exp(e) -> bf16
                    et = wp.tile([128, T, 4], F32, tag="et")
                    nc.vector.tensor_tensor(
                        out=et[:],
                        in0=hsrc[:, :, fo : fo + 4],
                        in1=hsrc[:, :, fo + 4 : fo + 8],
                        op=mybir.AluOpType.add,
                    )
                    nc.vector.tensor_tensor(
                        out=et[:], in0=et[:], in1=pd[:, :, 0:4],
                        op=mybir.AluOpType.add,
                    )
                    nc.vector.tensor_tensor(
                        out=et[:], in0=et[:], in1=pd[:, :, 4:8],
                        op=mybir.AluOpType.add,
                    )
                    nc.vector.scalar_tensor_tensor(
                        out=et[:], in0=et[:], scalar=0.2, in1=et[:],
                        op0=mybir.AluOpType.mult, op1=mybir.AluOpType.max,
                    )
                    etb = wp.tile([128, T, 4], BF16, tag="etb")
                    nc.scalar.activation(
                        out=etb[:], in_=et[:], func=mybir.ActivationFunctionType.Exp
                    )
                    if use_dbg and l == 0 and g == 0:
                        hsf = wp.tile([128, rp], F32, tag="hsf")
                        nc.vector.tensor_copy(hsf[:], hsrc[:, 0, :])
                        nc.sync.dma_start(dbg_hs[:], hsf[:])

                    # msg = ee * h (per tile)
                    msg = mp.tile([128, T, fo + 4], BF16, tag="msg")
                    for t in range(T):
                        eslice = etb[:, t, :]
                        ee_b = bass.AP(
                            eslice.tensor, eslice.offset, list(eslice.ap) + [[0, C]]
                        )
                        nc.vector.tensor_tensor(
                            out=msg[:, t, 0:fo].rearrange("p (h c) -> p h c", h=HEADS),
                            in0=hsrc[:, t, 0:fo].rearrange("p (h c) -> p h c", h=HEADS),
                            in1=ee_b,
                            op=mybir.AluOpType.mult,
                        )
                    nc.vector.tensor_copy(msg[:, :, fo : fo + 4], etb[:])
                    for t in range(T):
                        lhsT = ssw[:, 128 * t : 128 * (t + 1)]
                        if split0:
                            nc.tensor.matmul(
                                pe[:, 0:fo], lhsT=lhsT, rhs=msg[:, t, 0:fo],
                                start=(t == 0), stop=(t == T - 1),
                            )
                            nc.tensor.matmul(
                                pdm[:, 64:68], lhsT=lhsT,
                                rhs=msg[:, t, fo : fo + 4],
                                start=(t == 0), stop=(t == T - 1),
                            )
                        else:
                            nc.tensor.matmul(
                                pe[:, 0 : fo + 4], lhsT=lhsT, rhs=msg[:, t, :],
                                start=(t == 0), stop=(t == T - 1),
                            )
                    if g > 0:
                        tail_work(g - 1)
                tail_work(nblk - 1)

            # ---- readout: per-graph partial sums via one-hot G matmul
            if use_dbg:
                nc.sync.dma_start(dbg_p[:], p_sb[:])
            yp = yqp.tile([128, 1], F32, tag="yq")
            for g in range(nblk):
                nc.tensor.matmul(
                    yp[:],
                    lhsT=g_sb[:, 128 * g : 128 * (g + 1)],
                    rhs=p_sb[:, g : g + 1],
                    start=(g == 0), stop=(g == nblk - 1),
                )
            ypart_sb = pp.tile([128, 1], F32, tag="ypart")
            nc.scalar.copy(ypart_sb[:], yp[:])
            nc.sync.dma_start(ypart_d[:], ypart_sb[:])
            nc.gpsimd.collective_compute(
                AG,
                mybir.AluOpType.bypass,
                replica_groups=rg,
                ins=[ypart_d[:]],
                outs=[yglob_d[:]],
            )
            ya = pp.tile([128, cfg.n_cores], F32, tag="ya")
            nc.sync.dma_start(
                ya[:], yglob_d.rearrange("(c g) one -> g (c one)", g=128)
            )
            if use_dbg:
                nc.sync.dma_start(dbg_ya[:], ya[:])
            ysum = pp.tile([128, 1], F32, tag="ysum")
            yjunk = pp.tile([128, cfg.n_cores], F32, tag="yjunk")
            nc.vector.scalar_tensor_tensor(
                out=yjunk[:], in0=ya[:], scalar=1.0,
                in1=ones_sb[:, 0:1].to_broadcast([128, cfg.n_cores]),
                op0=mybir.AluOpType.mult, op1=mybir.AluOpType.mult,
                accum_out=ysum[:],
            )
            yq = yqp.tile([cfg.gpc, 1], F32, tag="yq")
            nc.tensor.matmul(
                yq[:], lhsT=ysel_sb[:], rhs=ysum[:], start=True, stop=True
            )
            y_sb = pp.tile([cfg.gpc, 1], F32, tag="y_sb")
            nc.scalar.copy(y_sb[:], yq[:])
            nc.sync.dma_start(y_d[:], y_sb[:])

    nc.compile()
    return nc


# ------------------------------------------------------------------- driver

last_results = None  # BassKernelResults of the most recent run (for test.py)
_cache = {}


def _prepare(cfg, inputs):
    T, perm_pad, per_core = preprocess(cfg, np.asarray(inputs["edge_index"]))
    x = np.asarray(inputs["x"], dtype=np.float32)
    fcw = np.asarray(inputs["fcw"], dtype=np.float32)
    fcb = float(np.asarray(inputs["fcb"]).reshape(-1)[0])
    waugs = [
        make_waug(
            np.asarray(inputs[f"W{l + 1}"], np.float32),
            np.asarray(inputs[f"as{l + 1}"], np.float32),
            np.asarray(inputs[f"ad{l + 1}"], np.float32),
        )
        for l in range(cfg.n_layers)
    ]
    biases = [
        np.tile(np.asarray(inputs[f"b{l + 1}"], np.float32)[None, :], (128, 1))
        for l in range(cfg.n_layers)
    ]
    n = cfg.n_nodes
    nodes = np.arange(n)
    fcw_node_full = fcw.reshape(cfg.npg, 64)[nodes % cfg.npg]  # [N, 64]
    graph_of = nodes // cfg.npg

    in_maps = []
    for c in range(cfg.n_cores):
        # nodes mapped to this core, by padded-local position
        loc = perm_pad[nodes] - c * cfg.npc_pad
        m = (loc >= 0) & (loc < cfg.npc_pad)
        vsel = nodes[m]
        lsel = loc[m]
        xT0 = np.zeros((cfg.in_feat, cfg.npc_pad), np.float32)
        xT0[:, lsel] = x[vsel].T
        fcwn = np.zeros((cfg.npc_pad, 64), np.float32)
        fcwn[lsel] = fcw_node_full[vsel]
        gmat = np.zeros((128, 128 * cfg.nblk), np.float32)
        for v, lo in zip(vsel, lsel):
            w, s = lo // 128, lo % 128
            gmat[s, w * 128 + graph_of[v]] = 1.0
        ysel = np.zeros((128, cfg.gpc), np.float32)
        for g in range(cfg.gpc):
            ysel[c * cfg.gpc + g, g] = 1.0
        m_ = dict(
            xT0=np.ascontiguousarray(xT0.astype(ml_dtypes.bfloat16)),
            gidx=per_core[c]["gidx"],
            st=per_core[c]["st"],
            ss=per_core[c]["s"],
            gmat=np.ascontiguousarray(gmat),
            ysel=np.ascontiguousarray(ysel),
            fcwn=np.ascontiguousarray(fcwn),
        )
        for l in range(cfg.n_layers):
            m_[f"waug{l}"] = waugs[l]
            m_[f"bias{l}"] = biases[l]
        in_maps.append(m_)
    return T, in_maps, fcb


def _ensure_ntff_hook():
    """Shim antenv.axon_hooks (absent in this image) so BASS_TRACE works."""
    try:
        from antenv.axon_hooks import get_axon_ntff_profile_hook  # noqa: F401

        return
    except ImportError:
        pass
    try:
        import types

        import antenv

        mod = types.ModuleType("antenv.axon_hooks")
        holder = [None]
        mod.set_axon_ntff_profile_hook = lambda h: holder.__setitem__(0, h)
        mod.get_axon_ntff_profile_hook = lambda: holder[0]
        sys.modules["antenv.axon_hooks"] = mod
        antenv.axon_hooks = mod
        from trn_agent_boot.trn_boot import _ntff_profile_via_ctypes

        h = _ntff_profile_via_ctypes("/opt/axon/libaxon_pjrt.so")
        if h is not None:
            holder[0] = h
    except Exception:
        pass


def run(cfg, inputs, trace=False, dbg=False):
    global last_results
    if trace or os.environ.get("BASS_TRACE"):
        _ensure_ntff_hook()
    T, in_maps, fcb = _prepare(cfg, inputs)
    key = (cfg.n_nodes, T, dbg)
    if key not in _cache:
        _cache[key] = build_kernel(cfg, T, dbg=dbg)
    nc = _cache[key]
    res = run_bass_kernel_spmd(
        nc, in_maps, core_ids=list(range(cfg.n_cores)), trace=trace
    )
    last_results = res
    y = np.concatenate([r["y"].reshape(-1) for r in res.results])
    return (y.reshape(-1, 1) + fcb).astype(np.float32)


def kernel(**inputs) -> np.ndarray:
    cfg = default_cfg()
    return run(cfg, inputs)
